# revision 38
# baseline (speedup 1.0000x reference)
"""Trainium2 Bass kernel for nn_Block_22832046145821 (dense_mlp).

Reference computation (B=256, D0=16, D1=32, D2=64, D_FFN=2048):
    x1 = x.reshape(B, D0, F1)                    F1 = D1*D2 = 2048
    u  = mlp1_i(x1[:, i, :]) for each i          (16 independent MLPs, hidden 2048)
    x2 = x.transpose(0,2,1,3).reshape(B, D1, F2) F2 = D0*D2 = 1024
    v  = mlp2_j(x2[:, j, :]) for each j          (32 independent MLPs, hidden 2048)
    out = x + 0.5*(u + v)

Sharding: expert-parallel across 8 cores. Core c owns mlp1 experts
{2c, 2c+1} and mlp2 experts {4c..4c+3}; every core sees the full batch,
keeping per-core weight traffic at the 1/8 minimum.

fp8 DoubleRow pipeline (2x bf16 PE throughput):
    Weights and activations are float8e4m3. The host pre-scales W by 64
    (exact power of two; clears the e4m3 min-normal 2^-6 for the 0.02-scale
    weights) and packs stationary slabs pair-major: one DoubleRow matmul
    contracts 256 rows (two 128-row subtiles), stationary [128, 2, 128] x
    moving [128, 2, B], fp32 PSUM accumulation. The 1/64 compensation folds
    into the activation scale (gelu(ps/64 + b0)) and the epilogue
    (0.5/64, with 0.5*b1 pre-scaled on host). gelu outputs are written
    directly as fp8 for GEMM2's moving operand; outputs return as bf16.
    Exact rel-err vs the fp32 reference: 1.63e-2 (e4m3 mantissa rounding of
    W0/W1/x/h; measured on the real seed-0 inputs, gate is 2e-2).

Why this schedule (from profile iteration; HW ~141us/core):
- Steady state is LDWEIGHTS-bound: each DR matmul needs a 256-row weight
  load at 1 row/cycle -> 109ns cadence, 1024 matmuls -> ~112us PE floor.
  The DMA stream (33.6MB weights + 6MB io at ~275GB/s single-queue) and
  SBUF port bandwidth (DMA writes + LW reads + moving reads ~9B/ns/
  partition) sit at essentially the same floor, so the three are balanced.
- ALL weight slabs ride the sync queue, single 512KB descriptors: one
  in-order queue delivers bytes in exactly need-order. Splitting across
  queues raises aggregate DMA but throttles LDWEIGHTS via SBUF-port
  contention and breaks arrival order - measured strictly worse.
- xt/bias/out ride the scalar queue (small, off the critical stream).
- Accumulation phases use 4 PSUM banks (pool of 8) so phase k+1's matmuls
  overlap phase k's activation drain; each region owns a whole bank
  (matmul start=True clears has_written bank-wide).
- A dummy gelu+identity activation at program start pulls the 2.6us
  ACT_TABLE_LOAD into the DMA fill instead of the first PSUM drain.
- The next expert's xt/bias loads are emitted before this expert's GEMM2
  so the scalar queue prefetches across the expert boundary.
- Assembled with bacc.Bacc: finalize() runs generate_event_semaphores for
  the 1-wait-per-64B-instruction TRN2 encoding.
"""

import os
import sys
from concurrent.futures import ThreadPoolExecutor

import numpy as np

try:
    import concourse.bass as bass
except ImportError:  # pragma: no cover
    sys.path.insert(0, "/opt/trn_rl_repo")
    import concourse.bass as bass

import ml_dtypes
import concourse.mybir as mybir
from concourse import bacc
from concourse.bass_utils import run_bass_kernel_spmd
from concourse.tile import TileContext

B, D0, D1, D2 = 256, 16, 32, 64
DF = 2048
F1 = D1 * D2  # 2048
F2 = D0 * D2  # 1024
NCORES = 8
E1 = D0 // NCORES  # 2 mlp1 experts per core
E2 = D1 // NCORES  # 4 mlp2 experts per core
KT = DF // 128  # 16 k-tiles
WS = 64.0  # host weight pre-scale (power of two, exact)

BF16 = mybir.dt.bfloat16
FP8 = mybir.dt.float8e4
F32 = mybir.dt.float32
NPBF16 = ml_dtypes.bfloat16
NPFP8 = ml_dtypes.float8_e4m3

GELU = mybir.ActivationFunctionType.Gelu
IDENT = mybir.ActivationFunctionType.Identity
DR = mybir.MatmulPerfMode.DoubleRow

_PROGRAM = None


class _Ring:
    """Explicit round-robin ring of SBUF tiles."""

    def __init__(self, pool, shape, dtype, n, name):
        self.tiles = [
            pool.tile(shape, dtype, name=f"{name}{i}", tag=f"{name}{i}")
            for i in range(n)
        ]
        self.idx = 0

    def acquire(self):
        i = self.idx % len(self.tiles)
        self.idx += 1
        return self.tiles[i]


def _emit_loads(nc, rings, spec):
    """Input DMAs for one expert: xT (one transfer) + combined bias, on the
    ACT engine's DGE queue so SP stays free for weight-slab issue."""
    xring, wring, w2ring, hring, bpool, oring, pspool = rings
    xt_dram, bb_dram, e, F, tag = (
        spec["xt"], spec["bb"], spec["e"], spec["F"], spec["tag"])
    FT = F // 128
    xt = xring.acquire()
    nc.scalar.dma_start(out=xt[:, :FT, :], in_=xt_dram[e])
    bb = bpool.tile([128, KT + FT], F32, tag=f"bb_{tag}_{e}")
    nc.scalar.dma_start(out=bb[:], in_=bb_dram[e])
    return {"xt": xt, "b0": bb[:, :KT], "b1": bb[:, KT:KT + FT]}


def _emit_expert_mlp(nc, rings, spec, loads, next_loads_fn, last=False):
    """One expert MLP: [F] -> gelu -> [DF] -> [F], batch B, transposed
    layout, fp8 DoubleRow matmuls.

    spec tensors (per expert e), packing done on host:
      xt:  [E, 128, F//128, B]     fp8  x.T, partition-major
      w0t: [E, F//256, 128, 2, DF] fp8  64*W0.T, pair-major slabs
      w1t: [E, KT//2, 128, 2, F]   fp8  64*W1.T, pair-major slabs
      bb:  [E, 128, KT + F//128]   f32  [b0 | 0.5*b1], partition-major
      out: [E, F//512, 128, 4, B]  bf16 (0.5*y.T, phase-batched)
    """
    xring, wring, w2ring, hring, bpool, oring, pspool = rings
    w0t_dram, w1t_dram, out_dram, e, F = (
        spec["w0t"], spec["w1t"], spec["out"], spec["e"], spec["F"])
    FT = F // 128   # 16 (mlp1) or 8 (mlp2)
    G = F // 256    # GEMM1 contraction pair-groups: 8 or 4
    H = KT // 2     # GEMM2 contraction pair-groups: 8
    xt, b0, b1 = loads["xt"], loads["b0"], loads["b1"]
    ht = hring.acquire()

    # GEMM1: pair-major fp8 slabs [128, 2, DF], resident across all phases.
    slabs = []
    for g in range(G):
        slab = wring.acquire()
        nc.sync.dma_start(out=slab[:], in_=w0t_dram[e, g])
        slabs.append(slab)
    for q in range(KT // 4):  # 4 phases x 4 PSUM banks
        ps = [pspool.tile([128, 512], F32, tag="ps", name=f"ps{i}") for i in range(4)]
        for g in range(G):
            for k4 in range(4):
                kt = q * 4 + k4
                nc.tensor.matmul(
                    ps[k4][:, :B],
                    lhsT=slabs[g][:, :, kt * 128:(kt + 1) * 128],
                    rhs=xt[:, 2 * g:2 * g + 2, :],
                    start=(g == 0),
                    stop=(g == G - 1),
                    perf_mode=DR,
                )
        for k4 in range(4):
            kt = q * 4 + k4
            nc.scalar.activation(
                ht[:, kt, :], ps[k4][:, :B], GELU, bias=b0[:, kt:kt + 1],
                scale=1.0 / WS,
            )

    # Prefetch the next expert's inputs now: the xt ring slot was released
    # by this expert's last GEMM1 matmul, so the load overlaps all of GEMM2.
    next_loads = next_loads_fn() if next_loads_fn is not None else None

    # GEMM2: fp8 slabs [128, 2, F] on the same sync queue (need-order).
    slabs2 = []
    for h in range(H):
        slab = w2ring.acquire()
        nc.sync.dma_start(out=slab[:, :, :F], in_=w1t_dram[e, h])
        slabs2.append(slab)
    for p in range(FT // 4):  # 4 (mlp1) or 2 (mlp2) phases x 4 banks
        ps = [pspool.tile([128, 512], F32, tag="ps", name=f"ps{i}")
              for i in range(4)]
        for h in range(H):
            for f4 in range(4):
                ft = p * 4 + f4
                nc.tensor.matmul(
                    ps[f4][:, :B],
                    lhsT=slabs2[h][:, :, ft * 128:(ft + 1) * 128],
                    rhs=ht[:, 2 * h:2 * h + 2, :],
                    start=(h == 0),
                    stop=(h == H - 1),
                    perf_mode=DR,
                )
        ot = oring.acquire()
        for f4 in range(4):
            ft = p * 4 + f4
            nc.scalar.activation(
                ot[:, f4, :], ps[f4][:, :B], IDENT,
                bias=b1[:, ft:ft + 1], scale=0.5 / WS,
            )
        if last and p == FT // 4 - 1:
            # Final phase: write per-tile so the drain overlaps the ACTs.
            for f4 in range(4):
                nc.scalar.dma_start(out=out_dram[e, p, :, f4], in_=ot[:, f4, :])
        else:
            nc.scalar.dma_start(out=out_dram[e, p], in_=ot[:])
    return next_loads


def _build_program():
    nc = bacc.Bacc()

    xt1 = nc.dram_tensor("xt1", [E1, 128, F1 // 128, B], FP8, kind="ExternalInput")
    w0t1 = nc.dram_tensor("w0t1", [E1, F1 // 256, 128, 2, DF], FP8,
                          kind="ExternalInput")
    w1t1 = nc.dram_tensor("w1t1", [E1, KT // 2, 128, 2, F1], FP8,
                          kind="ExternalInput")
    bb1 = nc.dram_tensor("bb1", [E1, 128, KT + F1 // 128], F32, kind="ExternalInput")
    xt2 = nc.dram_tensor("xt2", [E2, 128, F2 // 128, B], FP8, kind="ExternalInput")
    w0t2 = nc.dram_tensor("w0t2", [E2, F2 // 256, 128, 2, DF], FP8,
                          kind="ExternalInput")
    w1t2 = nc.dram_tensor("w1t2", [E2, KT // 2, 128, 2, F2], FP8,
                          kind="ExternalInput")
    bb2 = nc.dram_tensor("bb2", [E2, 128, KT + F2 // 128], F32, kind="ExternalInput")
    outU = nc.dram_tensor("outU", [E1, F1 // 512, 128, 4, B], BF16,
                          kind="ExternalOutput")
    outV = nc.dram_tensor("outV", [E2, F2 // 512, 128, 4, B], BF16,
                          kind="ExternalOutput")

    specs_u = [
        {"xt": xt1, "w0t": w0t1, "w1t": w1t1, "bb": bb1,
         "out": outU, "e": e, "F": F1, "tag": "u"}
        for e in range(E1)
    ]
    specs_v = [
        {"xt": xt2, "w0t": w0t2, "w1t": w1t2, "bb": bb2,
         "out": outV, "e": e, "F": F2, "tag": "v"}
        for e in range(E2)
    ]
    # Start with an mlp2 expert: its GEMM1 needs only 4 slabs (2MB), so the
    # cold-start fill stall is half as long as an mlp1 expert's.
    specs = [specs_v[0]] + specs_u + specs_v[1:]

    with TileContext(nc) as tc:
        with (
            tc.tile_pool(name="xp", bufs=1) as xpool,
            tc.tile_pool(name="wp", bufs=1) as wpool,
            tc.tile_pool(name="hp", bufs=1) as hpool,
            tc.tile_pool(name="bp", bufs=1) as bpool,
            tc.tile_pool(name="op", bufs=1) as opool,
            tc.tile_pool(name="pp", bufs=8, space="PSUM") as pspool,
        ):
            xring = _Ring(xpool, [128, F1 // 128, B], FP8, 2, "xt")
            wring = _Ring(wpool, [128, 2, DF], FP8, 16, "w")
            w2ring = _Ring(wpool, [128, 2, F1], FP8, 16, "w2")
            hring = _Ring(hpool, [128, KT, B], FP8, 2, "ht")
            oring = _Ring(opool, [128, 4, B], BF16, 4, "ot")
            rings = (xring, wring, w2ring, hring, bpool, oring, pspool)

            # Warm the ACT tables (gelu + identity) during the DMA fill so
            # the 2x1.28us ACT_TABLE_LOAD is off the first drain's path.
            warm = bpool.tile([128, 2], F32, tag="warm")
            nc.scalar.activation(warm[:, 0:1], warm[:, 1:2], GELU)
            nc.scalar.activation(warm[:, 0:1], warm[:, 1:2], IDENT)

            loads = _emit_loads(nc, rings, specs[0])
            for i, spec in enumerate(specs):
                if i + 1 < len(specs):
                    nl_fn = (lambda s=specs[i + 1]: _emit_loads(nc, rings, s))
                else:
                    nl_fn = None
                nxt = _emit_expert_mlp(nc, rings, spec, loads, nl_fn,
                                       last=(i + 1 == len(specs)))
                loads = nxt

    nc.finalize()
    return nc


def _get_program():
    global _PROGRAM
    if _PROGRAM is None:
        _PROGRAM = _build_program()
    return _PROGRAM


def _part_major(b, n_tiles):
    # [E, n_tiles*128] f32 -> [E, 128, n_tiles], partition-major bias layout
    e = b.shape[0]
    return np.ascontiguousarray(
        b.reshape(e, n_tiles, 128).transpose(0, 2, 1)).astype(np.float32)


def _pack_xt(xs):
    # [B, E, F] -> [E, 128, F//128, B] (partition-major xT), fp8
    Bn, En, Fn = xs.shape
    xt = xs.transpose(1, 2, 0).reshape(En, Fn // 128, 128, Bn)
    return np.ascontiguousarray(xt.transpose(0, 2, 1, 3)).astype(NPFP8)


def _pack_w(w):
    # [E, DF_out, F_in] (applied along F_in) -> pair-major stationary slabs
    # [E, F_in//256, 128, 2, DF_out] of 64*W.T in fp8:
    # element [e, g, p, i, k] = 64*W.T[e, g*256 + i*128 + p, k].
    En, DFo, Fi = w.shape
    wt = (w.transpose(0, 2, 1) * WS).reshape(En, Fi // 256, 2, 128, DFo)
    return np.ascontiguousarray(wt.transpose(0, 1, 3, 2, 4)).astype(NPFP8)


def _pack_core(c, x1, x2, W0_1, b0_1, W1_1, b1_1, W0_2, b0_2, W1_2, b1_2):
    i0, j0 = c * E1, c * E2
    s1, s2 = slice(i0, i0 + E1), slice(j0, j0 + E2)
    bb1 = np.concatenate(
        [_part_major(b0_1[s1], KT), _part_major(0.5 * b1_1[s1], F1 // 128)],
        axis=2)
    bb2 = np.concatenate(
        [_part_major(b0_2[s2], KT), _part_major(0.5 * b1_2[s2], F2 // 128)],
        axis=2)
    return {
        "xt1": _pack_xt(x1[:, s1, :]),
        "w0t1": _pack_w(W0_1[s1]),
        "w1t1": _pack_w(W1_1[s1]),
        "bb1": np.ascontiguousarray(bb1),
        "xt2": _pack_xt(x2[:, s2, :]),
        "w0t2": _pack_w(W0_2[s2]),
        "w1t2": _pack_w(W1_2[s2]),
        "bb2": np.ascontiguousarray(bb2),
    }


def run(inputs, trace=False):
    """Returns (out, BassKernelResults)."""
    x = np.asarray(inputs["x"], dtype=np.float32)
    x1 = x.reshape(B, D0, F1)
    x2 = np.ascontiguousarray(x.transpose(0, 2, 1, 3)).reshape(B, D1, F2)
    args = tuple(
        np.asarray(inputs[k], dtype=np.float32)
        for k in ("W0_1", "b0_1", "W1_1", "b1_1", "W0_2", "b0_2", "W1_2", "b1_2")
    )

    with ThreadPoolExecutor(max_workers=NCORES) as ex:
        in_maps = list(ex.map(lambda c: _pack_core(c, x1, x2, *args), range(NCORES)))
    nc = _get_program()
    res = run_bass_kernel_spmd(nc, in_maps, list(range(NCORES)), trace=trace)

    # [E, F//512, 128, 4, B] phase-batched -> [E, F, B]
    U = np.concatenate([r["outU"] for r in res.results], axis=0).astype(np.float32)
    V = np.concatenate([r["outV"] for r in res.results], axis=0).astype(np.float32)
    U = U.transpose(0, 1, 3, 2, 4).reshape(D0, F1, B)
    V = V.transpose(0, 1, 3, 2, 4).reshape(D1, F2, B)
    u_half = U.transpose(2, 0, 1).reshape(B, D0, D1, D2)
    v_half = V.transpose(2, 0, 1).reshape(B, D1, D0, D2).transpose(0, 2, 1, 3)
    out = x + u_half + v_half
    return np.ascontiguousarray(out, dtype=np.float32), res


def kernel(**inputs) -> np.ndarray:
    out, _ = run(inputs, trace=False)
    return out


# revision 44
# speedup vs baseline: 1.1756x; 1.1756x over previous
"""Trainium2 Bass kernel for nn_Block_22832046145821 (dense_mlp).

Reference computation (B=256, D0=16, D1=32, D2=64, D_FFN=2048):
    x1 = x.reshape(B, D0, F1)                    F1 = D1*D2 = 2048
    u  = mlp1_i(x1[:, i, :]) for each i          (16 independent MLPs, hidden 2048)
    x2 = x.transpose(0,2,1,3).reshape(B, D1, F2) F2 = D0*D2 = 1024
    v  = mlp2_j(x2[:, j, :]) for each j          (32 independent MLPs, hidden 2048)
    out = x + 0.5*(u + v)

Sharding: expert-parallel across 8 cores. Core c owns mlp1 experts
{2c, 2c+1} and mlp2 experts {4c..4c+3}; every core sees the full batch,
keeping per-core weight traffic at the 1/8 minimum.

fp8 DoubleRow pipeline (2x bf16 PE throughput):
    Weights and activations are float8e4m3. The host pre-scales W by 64
    (exact power of two; clears the e4m3 min-normal 2^-6 for the 0.02-scale
    weights) and packs stationary slabs pair-major: one DoubleRow matmul
    contracts 256 rows (two 128-row subtiles), stationary [128, 2, 128] x
    moving [128, 2, B], fp32 PSUM accumulation. The 1/64 compensation folds
    into the activation scale (gelu(ps/64 + b0)) and the epilogue
    (0.5/64, with 0.5*b1 pre-scaled on host). gelu outputs are written
    directly as fp8 for GEMM2's moving operand; outputs return as bf16.
    Exact rel-err vs the fp32 reference: 1.63e-2 (e4m3 mantissa rounding of
    W0/W1/x/h; measured on the real seed-0 inputs, gate is 2e-2).

Why this schedule (from profile iteration; HW ~141us/core):
- Steady state is LDWEIGHTS-bound: each DR matmul needs a 256-row weight
  load at 1 row/cycle -> 109ns cadence, 1024 matmuls -> ~112us PE floor.
  The DMA stream (33.6MB weights + 6MB io at ~275GB/s single-queue) and
  SBUF port bandwidth (DMA writes + LW reads + moving reads ~9B/ns/
  partition) sit at essentially the same floor, so the three are balanced.
- ALL weight slabs ride the sync queue, single 512KB descriptors: one
  in-order queue delivers bytes in exactly need-order. Splitting across
  queues raises aggregate DMA but throttles LDWEIGHTS via SBUF-port
  contention and breaks arrival order - measured strictly worse.
- xt/bias/out ride the scalar queue (small, off the critical stream).
- Accumulation phases use 4 PSUM banks (pool of 8) so phase k+1's matmuls
  overlap phase k's activation drain; each region owns a whole bank
  (matmul start=True clears has_written bank-wide).
- A dummy gelu+identity activation at program start pulls the 2.6us
  ACT_TABLE_LOAD into the DMA fill instead of the first PSUM drain.
- The next expert's xt/bias loads are emitted before this expert's GEMM2
  so the scalar queue prefetches across the expert boundary.
- Assembled with bacc.Bacc: finalize() runs generate_event_semaphores for
  the 1-wait-per-64B-instruction TRN2 encoding.
"""

import os
import sys
from concurrent.futures import ThreadPoolExecutor

import numpy as np

try:
    import concourse.bass as bass
except ImportError:  # pragma: no cover
    sys.path.insert(0, "/opt/trn_rl_repo")
    import concourse.bass as bass

import ml_dtypes
import concourse.mybir as mybir
from concourse import bacc
from concourse.bass_utils import run_bass_kernel_spmd
from concourse.tile import TileContext

B, D0, D1, D2 = 256, 16, 32, 64
DF = 2048
F1 = D1 * D2  # 2048
F2 = D0 * D2  # 1024
NCORES = 8
E1 = D0 // NCORES  # 2 mlp1 experts per core
E2 = D1 // NCORES  # 4 mlp2 experts per core
KT = DF // 128  # 16 k-tiles
WS = 64.0  # host weight pre-scale (power of two, exact)

BF16 = mybir.dt.bfloat16
FP8 = mybir.dt.float8e4
F32 = mybir.dt.float32
NPBF16 = ml_dtypes.bfloat16
NPFP8 = ml_dtypes.float8_e4m3

GELU = mybir.ActivationFunctionType.Gelu
IDENT = mybir.ActivationFunctionType.Identity
DR = mybir.MatmulPerfMode.DoubleRow

_PROGRAM = None


class _Ring:
    """Explicit round-robin ring of SBUF tiles."""

    def __init__(self, pool, shape, dtype, n, name):
        self.tiles = [
            pool.tile(shape, dtype, name=f"{name}{i}", tag=f"{name}{i}")
            for i in range(n)
        ]
        self.idx = 0

    def acquire(self):
        i = self.idx % len(self.tiles)
        self.idx += 1
        return self.tiles[i]


def _emit_loads(nc, rings, spec):
    """Input DMAs for one expert: xT (one transfer) + combined bias, on the
    ACT engine's DGE queue so SP stays free for weight-slab issue."""
    xring, wring, w2ring, hring, bpool, oring, pspool = rings
    xt_dram, bb_dram, e, F, tag = (
        spec["xt"], spec["bb"], spec["e"], spec["F"], spec["tag"])
    FT = F // 128
    xt = xring.acquire()
    nc.scalar.dma_start(out=xt[:, :FT, :], in_=xt_dram[e])
    bb = bpool.tile([128, KT + FT], F32, tag=f"bb_{tag}_{e}")
    nc.scalar.dma_start(out=bb[:], in_=bb_dram[e])
    return {"xt": xt, "b0": bb[:, :KT], "b1": bb[:, KT:KT + FT]}


def _emit_expert_mlp(nc, rings, spec, loads, next_loads_fn, last=False):
    """One expert MLP: [F] -> gelu -> [DF] -> [F], batch B, transposed
    layout, fp8 DoubleRow matmuls.

    spec tensors (per expert e), packing done on host:
      xt:  [E, 128, F//128, B]     fp8  x.T, partition-major
      w0t: [E, F//256, 128, 2, DF] fp8  64*W0.T, pair-major slabs
      w1t: [E, KT//2, 128, 2, F]   fp8  64*W1.T, pair-major slabs
      bb:  [E, 128, KT + F//128]   f32  [b0 | 0.5*b1], partition-major
      out: [E, F//512, 128, 4, B]  bf16 (0.5*y.T, phase-batched)
    """
    xring, wring, w2ring, hring, bpool, oring, pspool = rings
    w0t_dram, w1t_dram, out_dram, e, F = (
        spec["w0t"], spec["w1t"], spec["out"], spec["e"], spec["F"])
    FT = F // 128   # 16 (mlp1) or 8 (mlp2)
    G = F // 256    # GEMM1 contraction pair-groups: 8 or 4
    H = KT // 2     # GEMM2 contraction pair-groups: 8
    xt, b0, b1 = loads["xt"], loads["b0"], loads["b1"]
    ht = hring.acquire()

    # GEMM1: pair-major fp8 slabs [128, 2, DF], resident across all phases,
    # fetched as 1MB double-slab descriptors (measured ~306B/ns on the sync
    # queue vs ~275 with 512KB descriptors).
    slabs = []
    for gg in range(G // 2):
        t = wring.acquire()
        nc.sync.dma_start(out=t[:], in_=w0t_dram[e, gg])
        slabs.append(t[:, 0])
        slabs.append(t[:, 1])
    for q in range(KT // 4):  # 4 phases x 4 PSUM banks
        ps = [pspool.tile([128, 512], F32, tag="ps", name=f"ps{i}") for i in range(4)]
        for g in range(G):
            for k4 in range(4):
                kt = q * 4 + k4
                nc.tensor.matmul(
                    ps[k4][:, :B],
                    lhsT=slabs[g][:, :, kt * 128:(kt + 1) * 128],
                    rhs=xt[:, 2 * g:2 * g + 2, :],
                    start=(g == 0),
                    stop=(g == G - 1),
                    perf_mode=DR,
                )
        for k4 in range(4):
            kt = q * 4 + k4
            nc.scalar.activation(
                ht[:, kt, :], ps[k4][:, :B], GELU, bias=b0[:, kt:kt + 1],
                scale=1.0 / WS,
            )

    # Prefetch the next expert's inputs now: the xt ring slot was released
    # by this expert's last GEMM1 matmul, so the load overlaps all of GEMM2.
    next_loads = next_loads_fn() if next_loads_fn is not None else None

    # GEMM2: fp8 slabs [128, 2, F] on the same sync queue (need-order),
    # double-slab descriptors likewise.
    slabs2 = []
    for hh in range(H // 2):
        t = w2ring.acquire()
        nc.sync.dma_start(out=t[:, :, :, :F], in_=w1t_dram[e, hh])
        slabs2.append(t[:, 0])
        slabs2.append(t[:, 1])
    for p in range(FT // 4):  # 4 (mlp1) or 2 (mlp2) phases x 4 banks
        ps = [pspool.tile([128, 512], F32, tag="ps", name=f"ps{i}")
              for i in range(4)]
        for h in range(H):
            for f4 in range(4):
                ft = p * 4 + f4
                nc.tensor.matmul(
                    ps[f4][:, :B],
                    lhsT=slabs2[h][:, :, ft * 128:(ft + 1) * 128],
                    rhs=ht[:, 2 * h:2 * h + 2, :],
                    start=(h == 0),
                    stop=(h == H - 1),
                    perf_mode=DR,
                )
        ot = oring.acquire()
        for f4 in range(4):
            ft = p * 4 + f4
            nc.scalar.activation(
                ot[:, f4, :], ps[f4][:, :B], IDENT,
                bias=b1[:, ft:ft + 1], scale=0.5 / WS,
            )
        if last and p == FT // 4 - 1:
            # Final phase: write per-tile so the drain overlaps the ACTs.
            for f4 in range(4):
                nc.scalar.dma_start(out=out_dram[e, p, :, f4], in_=ot[:, f4, :])
        else:
            nc.scalar.dma_start(out=out_dram[e, p], in_=ot[:])
    return next_loads


def _build_program():
    nc = bacc.Bacc()

    xt1 = nc.dram_tensor("xt1", [E1, 128, F1 // 128, B], FP8, kind="ExternalInput")
    w0t1 = nc.dram_tensor("w0t1", [E1, F1 // 512, 128, 2, 2, DF], FP8,
                          kind="ExternalInput")
    w1t1 = nc.dram_tensor("w1t1", [E1, KT // 4, 128, 2, 2, F1], FP8,
                          kind="ExternalInput")
    bb1 = nc.dram_tensor("bb1", [E1, 128, KT + F1 // 128], F32, kind="ExternalInput")
    xt2 = nc.dram_tensor("xt2", [E2, 128, F2 // 128, B], FP8, kind="ExternalInput")
    w0t2 = nc.dram_tensor("w0t2", [E2, F2 // 512, 128, 2, 2, DF], FP8,
                          kind="ExternalInput")
    w1t2 = nc.dram_tensor("w1t2", [E2, KT // 4, 128, 2, 2, F2], FP8,
                          kind="ExternalInput")
    bb2 = nc.dram_tensor("bb2", [E2, 128, KT + F2 // 128], F32, kind="ExternalInput")
    outU = nc.dram_tensor("outU", [E1, F1 // 512, 128, 4, B], BF16,
                          kind="ExternalOutput")
    outV = nc.dram_tensor("outV", [E2, F2 // 512, 128, 4, B], BF16,
                          kind="ExternalOutput")

    specs_u = [
        {"xt": xt1, "w0t": w0t1, "w1t": w1t1, "bb": bb1,
         "out": outU, "e": e, "F": F1, "tag": "u"}
        for e in range(E1)
    ]
    specs_v = [
        {"xt": xt2, "w0t": w0t2, "w1t": w1t2, "bb": bb2,
         "out": outV, "e": e, "F": F2, "tag": "v"}
        for e in range(E2)
    ]
    # Start with an mlp2 expert: its GEMM1 needs only 4 slabs (2MB), so the
    # cold-start fill stall is half as long as an mlp1 expert's.
    specs = [specs_v[0]] + specs_u + specs_v[1:]

    with TileContext(nc) as tc:
        with (
            tc.tile_pool(name="xp", bufs=1) as xpool,
            tc.tile_pool(name="wp", bufs=1) as wpool,
            tc.tile_pool(name="hp", bufs=1) as hpool,
            tc.tile_pool(name="bp", bufs=1) as bpool,
            tc.tile_pool(name="op", bufs=1) as opool,
            tc.tile_pool(name="pp", bufs=8, space="PSUM") as pspool,
        ):
            xring = _Ring(xpool, [128, F1 // 128, B], FP8, 2, "xt")
            wring = _Ring(wpool, [128, 2, 2, DF], FP8, 10, "w")
            w2ring = _Ring(wpool, [128, 2, 2, F1], FP8, 10, "w2")
            hring = _Ring(hpool, [128, KT, B], FP8, 2, "ht")
            oring = _Ring(opool, [128, 4, B], BF16, 4, "ot")
            rings = (xring, wring, w2ring, hring, bpool, oring, pspool)

            # Warm the ACT tables (gelu + identity) during the DMA fill so
            # the 2x1.28us ACT_TABLE_LOAD is off the first drain's path.
            warm = bpool.tile([128, 2], F32, tag="warm")
            nc.scalar.activation(warm[:, 0:1], warm[:, 1:2], GELU)
            nc.scalar.activation(warm[:, 0:1], warm[:, 1:2], IDENT)

            loads = _emit_loads(nc, rings, specs[0])
            for i, spec in enumerate(specs):
                if i + 1 < len(specs):
                    nl_fn = (lambda s=specs[i + 1]: _emit_loads(nc, rings, s))
                else:
                    nl_fn = None
                nxt = _emit_expert_mlp(nc, rings, spec, loads, nl_fn,
                                       last=(i + 1 == len(specs)))
                loads = nxt

    nc.finalize()
    return nc


def _get_program():
    global _PROGRAM
    if _PROGRAM is None:
        _PROGRAM = _build_program()
    return _PROGRAM


def _part_major(b, n_tiles):
    # [E, n_tiles*128] f32 -> [E, 128, n_tiles], partition-major bias layout
    e = b.shape[0]
    return np.ascontiguousarray(
        b.reshape(e, n_tiles, 128).transpose(0, 2, 1)).astype(np.float32)


def _pack_xt(xs):
    # [B, E, F] -> [E, 128, F//128, B] (partition-major xT), fp8
    Bn, En, Fn = xs.shape
    xt = xs.transpose(1, 2, 0).reshape(En, Fn // 128, 128, Bn)
    return np.ascontiguousarray(xt.transpose(0, 2, 1, 3)).astype(NPFP8)


def _pack_w(w):
    # [E, DF_out, F_in] (applied along F_in) -> pair-major stationary
    # double-slabs [E, F_in//512, 128, 2, 2, DF_out] of 64*W.T in fp8:
    # element [e, gg, p, j, i, k] = 64*W.T[e, gg*512 + j*256 + i*128 + p, k].
    En, DFo, Fi = w.shape
    wt = (w.transpose(0, 2, 1) * WS).reshape(En, Fi // 512, 2, 2, 128, DFo)
    return np.ascontiguousarray(wt.transpose(0, 1, 4, 2, 3, 5)).astype(NPFP8)


def _pack_core(c, x1, x2, W0_1, b0_1, W1_1, b1_1, W0_2, b0_2, W1_2, b1_2):
    i0, j0 = c * E1, c * E2
    s1, s2 = slice(i0, i0 + E1), slice(j0, j0 + E2)
    bb1 = np.concatenate(
        [_part_major(b0_1[s1], KT), _part_major(0.5 * b1_1[s1], F1 // 128)],
        axis=2)
    bb2 = np.concatenate(
        [_part_major(b0_2[s2], KT), _part_major(0.5 * b1_2[s2], F2 // 128)],
        axis=2)
    return {
        "xt1": _pack_xt(x1[:, s1, :]),
        "w0t1": _pack_w(W0_1[s1]),
        "w1t1": _pack_w(W1_1[s1]),
        "bb1": np.ascontiguousarray(bb1),
        "xt2": _pack_xt(x2[:, s2, :]),
        "w0t2": _pack_w(W0_2[s2]),
        "w1t2": _pack_w(W1_2[s2]),
        "bb2": np.ascontiguousarray(bb2),
    }


def run(inputs, trace=False):
    """Returns (out, BassKernelResults)."""
    x = np.asarray(inputs["x"], dtype=np.float32)
    x1 = x.reshape(B, D0, F1)
    x2 = np.ascontiguousarray(x.transpose(0, 2, 1, 3)).reshape(B, D1, F2)
    args = tuple(
        np.asarray(inputs[k], dtype=np.float32)
        for k in ("W0_1", "b0_1", "W1_1", "b1_1", "W0_2", "b0_2", "W1_2", "b1_2")
    )

    with ThreadPoolExecutor(max_workers=NCORES) as ex:
        in_maps = list(ex.map(lambda c: _pack_core(c, x1, x2, *args), range(NCORES)))
    nc = _get_program()
    res = run_bass_kernel_spmd(nc, in_maps, list(range(NCORES)), trace=trace)

    # [E, F//512, 128, 4, B] phase-batched -> [E, F, B]
    U = np.concatenate([r["outU"] for r in res.results], axis=0).astype(np.float32)
    V = np.concatenate([r["outV"] for r in res.results], axis=0).astype(np.float32)
    U = U.transpose(0, 1, 3, 2, 4).reshape(D0, F1, B)
    V = V.transpose(0, 1, 3, 2, 4).reshape(D1, F2, B)
    u_half = U.transpose(2, 0, 1).reshape(B, D0, D1, D2)
    v_half = V.transpose(2, 0, 1).reshape(B, D1, D0, D2).transpose(0, 2, 1, 3)
    out = x + u_half + v_half
    return np.ascontiguousarray(out, dtype=np.float32), res


def kernel(**inputs) -> np.ndarray:
    out, _ = run(inputs, trace=False)
    return out
